# Initial kernel scaffold
#
"""Trainium2 Bass kernel for nn_AugmentPipe (gated flips / 90-degree rots /
reflect-pad integer translation), data-parallel over the batch on 8 cores.

The whole pipeline is a per-sample separable gather:
    out[y, x, c] = in[a[y], b[x], c]            (no transpose), or
    out[y, x, c] = in[a[x], b[y], c]            (rot 90/270)
where a, b are per-sample index vectors and the transpose flag comes from
rot_w. All per-sample control (flips, rotation, translation) is folded into
(a, b, transpose) on the host; the device program is identical for every
sample so one SPMD NEFF serves all 8 cores:

  1. dma_gather: rows in[a[k], :] -> SBUF (arbitrary row map, int16 idxs)
  2. column gather by b on DVE as 4 fixed copies (asc/desc main, asc/desc
     edge) whose source/dest element offsets are per-image registers loaded
     from a parameter table; b is always one +-1 main run (>=224) plus at
     most one +-1 edge run (<=32) from reflection padding, so padded
     fixed-length copies + overwrite order realize any b
  3. PE fp32 transpose (exact pass-through) of the gathered tile, always
  4. two cond-predicated DMA stores: untransposed or transposed result
"""
import sys

for _p in ("/opt/trn_rl_repo",):
    if _p not in sys.path:
        sys.path.insert(0, _p)

import numpy as np

N_CORES = 8
N, H, W, C = 128, 256, 256, 3
PER_CORE = N // N_CORES
ROW_ELEMS = W * C  # 768
PAD = 96  # 32 pixels of slack around each data block (elements)

# M1 (gather target) free-dim layout, in elements:
#   [96 lead pad][768 h0][768 h1][96 tail pad]  -> width 1728
M1_LEAD = PAD
M1_HSTRIDE = ROW_ELEMS
M1_W = PAD + 2 * ROW_ELEMS + PAD

# N (column-gathered) free-dim layout: [96 lead][768 h0][96 shared pad]
# [768 h1][96 tail][edge dump]. The dump must cover BOTH h-windows of the
# 2-block edge copy (stride 864) plus the 96-elem window itself -> 3456.
N_LEAD = PAD
N_HSTRIDE = ROW_ELEMS + PAD  # 864
N_DUMP = PAD + 2 * ROW_ELEMS + PAD + ROW_ELEMS + PAD  # edge dump start: 2496
N_W = N_DUMP + N_HSTRIDE + PAD  # 2496 + 864 + 96 = 3456

EDGE_PIX = 32
NPARAM = 7  # per-image int32 params: main src/dst, edge src/dst, R, 2 conds
# param layout: [5*PER_CORE offsets+R][2*PER_CORE store conds]


def _derive_maps(xflip_w, xflip_gate, yflip_w, yflip_gate, rot_w, rot_gate,
                 trans_w, trans_gate):
    """Replicate the reference gate logic; return (a[N,256], b[N,256], tr[N])."""
    f32 = np.float32
    n = xflip_w.shape[0]
    wx = np.where(np.asarray(xflip_gate).reshape(n) < f32(1.0),
                  np.asarray(xflip_w).reshape(n), 0)
    wy = np.where(np.asarray(yflip_gate).reshape(n) < f32(1.0),
                  np.asarray(yflip_w).reshape(n), 0)
    rw = np.where(np.asarray(rot_gate).reshape(n) < f32(1.0),
                  np.asarray(rot_w).reshape(n), 0)
    tw = np.asarray(trans_w, dtype=np.float32).reshape(2, n) * f32(2.0) - f32(1.0)
    tg = np.asarray(trans_gate).reshape(n)
    tw = np.where(tg[None, :] < f32(1.0), tw, f32(0.0)).astype(np.float32)
    tx = np.round((tw[0] * f32(W)) * f32(0.125)).astype(np.int32)
    ty = np.round((tw[1] * f32(H)) * f32(0.125)).astype(np.int32)

    idx = np.arange(W)
    xi = (W - 1) - np.abs((W - 1) - (idx[None, :] - tx[:, None]) % (2 * W - 2))
    yi = (H - 1) - np.abs((H - 1) - (idx[None, :] + ty[:, None]) % (2 * H - 2))

    xftot = (wx == 1) ^ ((rw == 1) | (rw == 2))
    yftot = (wy == 1) ^ ((rw == 2) | (rw == 3))
    tr = (rw == 1) | (rw == 3)

    a = np.where(tr[:, None], xi, yi)
    a = np.where(yftot[:, None], (H - 1) - a, a)
    b = np.where(tr[:, None], yi, xi)
    b = np.where(xftot[:, None], (W - 1) - b, b)
    return a.astype(np.int64), b.astype(np.int64), tr


def _fit_template(b):
    """Fit b (one +-1 main run >=224 plus <=1 edge run <=32) to the fixed
    4-copy template; return the 8 int32 element offsets
    [m_asc_src, m_asc_dst, m_desc_src, m_desc_dst,
     e_asc_src, e_asc_dst, e_desc_src, e_desc_dst]."""
    d = np.diff(b)
    assert np.all(np.abs(d) == 1), b
    change = np.nonzero(d[1:] != d[:-1])[0]
    assert len(change) <= 1, b
    if len(change) == 0:
        runs = [(0, W, int(d[0]))]
    else:
        # the pivot position can belong to either run; pick the split whose
        # short run is <= EDGE_PIX
        c0 = int(change[0])
        runs = None
        for cut in (c0 + 1, c0 + 2):
            r = [(0, cut, int(d[0])), (cut, W, int(d[cut]))]
            lens = sorted(e - s for s, e, _ in r)
            if lens[0] <= EDGE_PIX and lens[1] >= W - EDGE_PIX:
                runs = r
                break
        assert runs is not None, (b, c0)
    if len(runs) == 1:
        main, edge = runs[0], None
    else:
        r0, r1 = runs
        if (r0[1] - r0[0]) >= (r1[1] - r1[0]):
            main, edge = r0, r1
        else:
            main, edge = r1, r0
    mp, mq, md = main
    assert mq - mp >= W - EDGE_PIX, (b, runs)

    # main direction decides the branch: R=0 -> asc main + desc edge,
    # R=1 -> desc main + asc edge
    R = 0 if md == 1 else 1
    m_src = M1_LEAD + 3 * int(b[mp])
    m_dst = N_LEAD + 3 * mp

    if edge is not None:
        ep, eq, ed = edge
        assert eq - ep <= EDGE_PIX and ed == -md, (b, runs)
        if ep == 0:
            wstart = eq - EDGE_PIX  # head edge: window [eq-32, eq)
        else:
            assert eq == W, (b, runs)
            wstart = ep             # tail edge: window [ep, ep+32)
        v0 = int(b[ep]) + ed * (wstart - ep)  # value at window start
        e_src = M1_LEAD + 3 * v0
        e_dst = N_LEAD + 3 * wstart
        assert e_src >= 0 and e_dst >= 0, (b, runs, e_src, e_dst)
    else:
        # taken branch's edge copy still runs; point it at the dump
        e_src = M1_LEAD if md == -1 else M1_LEAD + 3 * (EDGE_PIX - 1)
        e_dst = N_DUMP

    return [m_src, m_dst, e_src, e_dst, R]


def _pack_gather_idx(a_core):
    """a_core: [PER_CORE, 256] row indices -> int16 [128, 16*PER_CORE] in
    dma_gather layout (index i at partition i%16, col i//16, replicated to
    all 8 gpsimd core partition groups)."""
    out = np.zeros((128, 16 * PER_CORE), np.int16)
    for img in range(PER_CORE):
        v = a_core[img].astype(np.int16)  # [256]
        blk = v.reshape(16, 16).T  # [p=i%16, s=i//16]
        for g in range(8):
            out[16 * g:16 * (g + 1), 16 * img:16 * (img + 1)] = blk
    return out


_NC_CACHE = {}


def _build_module(coresim_pads=False):
    key = ("nc", coresim_pads)
    if key in _NC_CACHE:
        return _NC_CACHE[key]
    import concourse.bacc as bacc
    import concourse.bass as bass
    import concourse.mybir as mybir
    import concourse.tile as tile
    from concourse.ap import AP

    DT = mybir.dt.float32
    nc = bacc.Bacc(None, num_swdge_queues=2)
    images = nc.dram_tensor("images", [PER_CORE, H, W, C], DT, kind="ExternalInput")
    identity_in = nc.dram_tensor("identity_in", [128, 128], DT, kind="ExternalInput")
    gidx = nc.dram_tensor("gidx", [128, 16 * PER_CORE], mybir.dt.int16,
                          kind="ExternalInput")
    params = nc.dram_tensor("params", [1, NPARAM * PER_CORE], mybir.dt.int32,
                            kind="ExternalInput")
    out = nc.dram_tensor("out", [PER_CORE, H, W, C], DT, kind="ExternalOutput")

    img_elems = H * W * C

    with tile.TileContext(nc) as tc:
        with (
            tc.tile_pool(name="const", bufs=1) as const_pool,
            tc.tile_pool(name="m1", bufs=5) as m1_pool,
            tc.tile_pool(name="ncg", bufs=4) as n_pool,
            tc.tile_pool(name="tt", bufs=4) as t_pool,
            tc.tile_pool(name="psum", bufs=8, space="PSUM") as psum_pool,
        ):
            ident = const_pool.tile([128, 128], DT)
            nc.sync.dma_start(ident[:], identity_in[:])
            idx_t = const_pool.tile([128, 16 * PER_CORE], mybir.dt.int16)
            nc.sync.dma_start(idx_t[:], gidx[:])
            par_t = const_pool.tile([1, NPARAM * PER_CORE], mybir.dt.int32)
            nc.sync.dma_start(par_t[:], params[:])

            dve = nc.vector.engine
            act = nc.scalar.engine
            sp = nc.sync.engine

            for i in range(PER_CORE):
                # --- 1. row gather: in[a[k], :] -> M1 ---
                m1 = m1_pool.tile([128, M1_W], DT, tag="m1")
                if coresim_pads:
                    # padded edge/main copies read into the lead/tail pads;
                    # the values only ever land in dump regions, but CoreSim
                    # requires every read to be initialized
                    nc.gpsimd.memset(m1[:, 0:M1_LEAD], 0.0)
                    nc.gpsimd.memset(m1[:, M1_W - PAD:M1_W], 0.0)
                src = AP(images[:].tensor, i * img_elems,
                         [[ROW_ELEMS, H], [1, ROW_ELEMS]])
                gout = m1[:, M1_LEAD:M1_LEAD + 2 * ROW_ELEMS].rearrange(
                    "p (h e) -> p h e", h=2)
                nc.gpsimd.dma_gather(
                    gout, src, idx_t[:, 16 * i:16 * (i + 1)],
                    num_idxs=H, num_idxs_reg=H, elem_size=ROW_ELEMS,
                    queue_num=i % 2, single_packet=False)

                # --- 2. column gather by b: M1 -> Ntile (4 reg-offset copies) ---
                ntile = n_pool.tile([128, N_W], DT, tag="ncg")
                m1t, ntt = m1[:].tensor, ntile[:].tensor
                p_m1 = [M1_W, 128]
                p_n = [N_W, 128]
                # per-image virtual registers; 5 per image (main src/dst,
                # edge src/dst, R flag), loaded per image pair. The R flag
                # branches ONLY the DVE stream: R=0 runs {asc main, desc
                # edge}, R=1 runs {desc main, asc edge} - halving DVE work
                # vs executing all four direction variants.
                if i % 2 == 0:
                    nload = min(2, PER_CORE - i) * 5
                    pair_regs = [nc.alloc_register(dve, f"cg{i}_{j}")
                                 for j in range(nload)]
                    nc.vector.reg_load(
                        pair_regs, par_t[0:1, 5 * i:5 * i + nload])
                dve_regs = pair_regs[5 * (i % 2):5 * (i % 2) + 5]
                with tc.If(bass.RuntimeValue(dve_regs[4]) < 1) as cmp:
                    nc.vector.tensor_copy(
                        AP(ntt, dve_regs[1], [p_n, [N_HSTRIDE, 2], [1, ROW_ELEMS]]),
                        AP(m1t, dve_regs[0], [p_m1, [M1_HSTRIDE, 2], [1, ROW_ELEMS]]))
                    nc.vector.tensor_copy(
                        AP(ntt, dve_regs[3], [p_n, [N_HSTRIDE, 2], [1, 3 * EDGE_PIX]]),
                        AP(m1t, dve_regs[2], [p_m1, [M1_HSTRIDE, 2], [-3, EDGE_PIX], [1, C]]))
                with cmp.Else():
                    nc.vector.tensor_copy(
                        AP(ntt, dve_regs[1], [p_n, [N_HSTRIDE, 2], [1, ROW_ELEMS]]),
                        AP(m1t, dve_regs[0], [p_m1, [M1_HSTRIDE, 2], [-3, W], [1, C]]))
                    nc.vector.tensor_copy(
                        AP(ntt, dve_regs[3], [p_n, [N_HSTRIDE, 2], [1, 3 * EDGE_PIX]]),
                        AP(m1t, dve_regs[2], [p_m1, [M1_HSTRIDE, 2], [1, 3 * EDGE_PIX]]))

                # --- 3. pixel transpose Ntile -> Ttile via PE (exact fp32) ---
                # 3 channel transposes interleave into one strided PSUM tile;
                # a single contiguous copy (on the otherwise-idle scalar
                # engine) moves each [128, 384] block out
                ttile = t_pool.tile([128, 2, ROW_ELEMS], DT, tag="tt")
                for hk in range(2):
                    for hu in range(2):
                        pt = psum_pool.tile([128, 3 * 128], DT, tag="pt")
                        ptt = pt[:].tensor
                        for c in range(C):
                            stat = AP(ntt, N_LEAD + hk * N_HSTRIDE + 3 * (hu * 128) + c,
                                      [p_n, [3, 128]])
                            nc.tensor.transpose(
                                AP(ptt, c, [[3 * 128, 128], [3, 128]]),
                                stat, ident[:])
                        t0 = 3 * (hk * 128)
                        nc.scalar.copy(ttile[:, hu, t0:t0 + 3 * 128], pt[:])

                # --- 4. predicated stores ---
                dram_out = AP(out[:].tensor, i * img_elems,
                              [[ROW_ELEMS, 128], [128 * ROW_ELEMS, 2], [1, ROW_ELEMS]])
                n_src = AP(ntt, N_LEAD, [p_n, [N_HSTRIDE, 2], [1, ROW_ELEMS]])
                cond_n_reg = nc.alloc_register(sp, f"cond_n_{i}")
                cond_t_reg = nc.alloc_register(sp, f"cond_t_{i}")
                cbase = 5 * PER_CORE + 2 * i
                nc.sync.reg_load([cond_n_reg, cond_t_reg],
                                 par_t[0:1, cbase:cbase + 2])
                cn = nc.sync.snap(cond_n_reg, min_val=0, max_val=1)
                ct = nc.sync.snap(cond_t_reg, min_val=0, max_val=1)
                nc.sync.dma_start(dram_out, n_src, cond=cn)
                nc.sync.dma_start(dram_out.copy(), ttile[:], cond=ct)

    nc.finalize()
    _NC_CACHE[key] = nc
    return nc


def _make_in_maps(images, a, b, tr):
    ident = np.eye(128, dtype=np.float32)
    in_maps = []
    for core in range(N_CORES):
        s = core * PER_CORE
        par = np.zeros((1, NPARAM * PER_CORE), np.int32)
        for i in range(PER_CORE):
            par[0, 5 * i:5 * i + 5] = _fit_template(b[s + i])
            par[0, 5 * PER_CORE + 2 * i] = 0 if tr[s + i] else 1
            par[0, 5 * PER_CORE + 2 * i + 1] = 1 if tr[s + i] else 0
        in_maps.append({
            "images": images[s:s + PER_CORE],
            "identity_in": ident,
            "gidx": _pack_gather_idx(a[s:s + PER_CORE]),
            "params": par,
        })
    return in_maps


def kernel(images, xflip_w, xflip_gate, yflip_w, yflip_gate, rot_w, rot_gate,
           trans_w, trans_gate):
    from concourse.bass_utils import run_bass_kernel_spmd

    images = np.ascontiguousarray(np.asarray(images, dtype=np.float32))
    a, b, tr = _derive_maps(xflip_w, xflip_gate, yflip_w, yflip_gate,
                            rot_w, rot_gate, trans_w, trans_gate)
    nc = _build_module()
    in_maps = _make_in_maps(images, a, b, tr)
    res = run_bass_kernel_spmd(nc, in_maps, list(range(N_CORES))).results
    return np.concatenate([res[c]["out"] for c in range(N_CORES)], axis=0)



# revision 1
# speedup vs baseline: 1.0450x; 1.0450x over previous
"""Trainium2 Bass kernel for nn_AugmentPipe (gated flips / 90-degree rots /
reflect-pad integer translation), data-parallel over the batch on 8 cores.

The whole pipeline is a per-sample separable gather:
    out[y, x, c] = in[a[y], b[x], c]            (no transpose), or
    out[y, x, c] = in[a[x], b[y], c]            (rot 90/270)
where a, b are per-sample index vectors and the transpose flag comes from
rot_w. All per-sample control (flips, rotation, translation) is folded into
(a, b, transpose) on the host; the device program is identical for every
sample so one SPMD NEFF serves all 8 cores:

  1. dma_gather: rows in[a[k], :] -> SBUF (arbitrary row map, int16 idxs)
  2. column gather by b on DVE as 4 fixed copies (asc/desc main, asc/desc
     edge) whose source/dest element offsets are per-image registers loaded
     from a parameter table; b is always one +-1 main run (>=224) plus at
     most one +-1 edge run (<=32) from reflection padding, so padded
     fixed-length copies + overwrite order realize any b
  3. PE fp32 transpose (exact pass-through) of the gathered tile, always
  4. two cond-predicated DMA stores: untransposed or transposed result
"""
import sys

for _p in ("/opt/trn_rl_repo",):
    if _p not in sys.path:
        sys.path.insert(0, _p)

import numpy as np

N_CORES = 8
N, H, W, C = 128, 256, 256, 3
PER_CORE = N // N_CORES
ROW_ELEMS = W * C  # 768
PAD = 96  # 32 pixels of slack around each data block (elements)

# M1 (gather target) free-dim layout, in elements:
#   [96 lead pad][768 h0][768 h1][96 tail pad]  -> width 1728
M1_LEAD = PAD
M1_HSTRIDE = ROW_ELEMS
M1_W = PAD + 2 * ROW_ELEMS + PAD

# N (column-gathered) free-dim layout: [96 lead][768 h0][96 shared pad]
# [768 h1][96 tail][edge dump]. The dump must cover BOTH h-windows of the
# 2-block edge copy (stride 864) plus the 96-elem window itself -> 3456.
N_LEAD = PAD
N_HSTRIDE = ROW_ELEMS + PAD  # 864
N_DUMP = PAD + 2 * ROW_ELEMS + PAD + ROW_ELEMS + PAD  # edge dump start: 2496
N_W = N_DUMP + N_HSTRIDE + PAD  # 2496 + 864 + 96 = 3456

EDGE_PIX = 32
NPARAM = 7  # per-image int32 params: main src/dst, edge src/dst, R, 2 conds
# param layout: [5*PER_CORE offsets+R][2*PER_CORE store conds]


def _derive_maps(xflip_w, xflip_gate, yflip_w, yflip_gate, rot_w, rot_gate,
                 trans_w, trans_gate):
    """Replicate the reference gate logic; return (a[N,256], b[N,256], tr[N])."""
    f32 = np.float32
    n = xflip_w.shape[0]
    wx = np.where(np.asarray(xflip_gate).reshape(n) < f32(1.0),
                  np.asarray(xflip_w).reshape(n), 0)
    wy = np.where(np.asarray(yflip_gate).reshape(n) < f32(1.0),
                  np.asarray(yflip_w).reshape(n), 0)
    rw = np.where(np.asarray(rot_gate).reshape(n) < f32(1.0),
                  np.asarray(rot_w).reshape(n), 0)
    tw = np.asarray(trans_w, dtype=np.float32).reshape(2, n) * f32(2.0) - f32(1.0)
    tg = np.asarray(trans_gate).reshape(n)
    tw = np.where(tg[None, :] < f32(1.0), tw, f32(0.0)).astype(np.float32)
    tx = np.round((tw[0] * f32(W)) * f32(0.125)).astype(np.int32)
    ty = np.round((tw[1] * f32(H)) * f32(0.125)).astype(np.int32)

    idx = np.arange(W)
    xi = (W - 1) - np.abs((W - 1) - (idx[None, :] - tx[:, None]) % (2 * W - 2))
    yi = (H - 1) - np.abs((H - 1) - (idx[None, :] + ty[:, None]) % (2 * H - 2))

    xftot = (wx == 1) ^ ((rw == 1) | (rw == 2))
    yftot = (wy == 1) ^ ((rw == 2) | (rw == 3))
    tr = (rw == 1) | (rw == 3)

    a = np.where(tr[:, None], xi, yi)
    a = np.where(yftot[:, None], (H - 1) - a, a)
    b = np.where(tr[:, None], yi, xi)
    b = np.where(xftot[:, None], (W - 1) - b, b)
    return a.astype(np.int64), b.astype(np.int64), tr


def _fit_template(b):
    """Fit b (one +-1 main run >=224 plus <=1 edge run <=32) to the fixed
    4-copy template; return the 8 int32 element offsets
    [m_asc_src, m_asc_dst, m_desc_src, m_desc_dst,
     e_asc_src, e_asc_dst, e_desc_src, e_desc_dst]."""
    d = np.diff(b)
    assert np.all(np.abs(d) == 1), b
    change = np.nonzero(d[1:] != d[:-1])[0]
    assert len(change) <= 1, b
    if len(change) == 0:
        runs = [(0, W, int(d[0]))]
    else:
        # the pivot position can belong to either run; pick the split whose
        # short run is <= EDGE_PIX
        c0 = int(change[0])
        runs = None
        for cut in (c0 + 1, c0 + 2):
            r = [(0, cut, int(d[0])), (cut, W, int(d[cut]))]
            lens = sorted(e - s for s, e, _ in r)
            if lens[0] <= EDGE_PIX and lens[1] >= W - EDGE_PIX:
                runs = r
                break
        assert runs is not None, (b, c0)
    if len(runs) == 1:
        main, edge = runs[0], None
    else:
        r0, r1 = runs
        if (r0[1] - r0[0]) >= (r1[1] - r1[0]):
            main, edge = r0, r1
        else:
            main, edge = r1, r0
    mp, mq, md = main
    assert mq - mp >= W - EDGE_PIX, (b, runs)

    # main direction decides the branch: R=0 -> asc main + desc edge,
    # R=1 -> desc main + asc edge
    R = 0 if md == 1 else 1
    m_src = M1_LEAD + 3 * int(b[mp])
    m_dst = N_LEAD + 3 * mp

    if edge is not None:
        ep, eq, ed = edge
        assert eq - ep <= EDGE_PIX and ed == -md, (b, runs)
        if ep == 0:
            wstart = eq - EDGE_PIX  # head edge: window [eq-32, eq)
        else:
            assert eq == W, (b, runs)
            wstart = ep             # tail edge: window [ep, ep+32)
        v0 = int(b[ep]) + ed * (wstart - ep)  # value at window start
        e_src = M1_LEAD + 3 * v0
        e_dst = N_LEAD + 3 * wstart
        assert e_src >= 0 and e_dst >= 0, (b, runs, e_src, e_dst)
    else:
        # taken branch's edge copy still runs; point it at the dump
        e_src = M1_LEAD if md == -1 else M1_LEAD + 3 * (EDGE_PIX - 1)
        e_dst = N_DUMP

    return [m_src, m_dst, e_src, e_dst, R]


def _pack_gather_idx(a_core):
    """a_core: [PER_CORE, 256] row indices -> int16 [128, 16*PER_CORE] in
    dma_gather layout (index i at partition i%16, col i//16, replicated to
    all 8 gpsimd core partition groups)."""
    out = np.zeros((128, 16 * PER_CORE), np.int16)
    for img in range(PER_CORE):
        v = a_core[img].astype(np.int16)  # [256]
        blk = v.reshape(16, 16).T  # [p=i%16, s=i//16]
        for g in range(8):
            out[16 * g:16 * (g + 1), 16 * img:16 * (img + 1)] = blk
    return out


_NC_CACHE = {}


def _build_module(coresim_pads=False):
    key = ("nc", coresim_pads)
    if key in _NC_CACHE:
        return _NC_CACHE[key]
    import concourse.bacc as bacc
    import concourse.bass as bass
    import concourse.mybir as mybir
    import concourse.tile as tile
    from concourse.ap import AP

    DT = mybir.dt.float32
    nc = bacc.Bacc(None, num_swdge_queues=2)
    images = nc.dram_tensor("images", [PER_CORE, H, W, C], DT, kind="ExternalInput")
    identity_in = nc.dram_tensor("identity_in", [128, 128], DT, kind="ExternalInput")
    gidx = nc.dram_tensor("gidx", [128, 16 * PER_CORE], mybir.dt.int16,
                          kind="ExternalInput")
    params = nc.dram_tensor("params", [1, NPARAM * PER_CORE], mybir.dt.int32,
                            kind="ExternalInput")
    out = nc.dram_tensor("out", [PER_CORE, H, W, C], DT, kind="ExternalOutput")

    img_elems = H * W * C

    with tile.TileContext(nc) as tc:
        with (
            tc.tile_pool(name="const", bufs=1) as const_pool,
            tc.tile_pool(name="m1", bufs=5) as m1_pool,
            tc.tile_pool(name="ncg", bufs=4) as n_pool,
            tc.tile_pool(name="tt", bufs=4) as t_pool,
            tc.tile_pool(name="psum", bufs=8, space="PSUM") as psum_pool,
        ):
            ident = const_pool.tile([128, 128], DT)
            nc.sync.dma_start(ident[:], identity_in[:])
            idx_t = const_pool.tile([128, 16 * PER_CORE], mybir.dt.int16)
            nc.sync.dma_start(idx_t[:], gidx[:])
            par_t = const_pool.tile([1, NPARAM * PER_CORE], mybir.dt.int32)
            nc.sync.dma_start(par_t[:], params[:])

            dve = nc.vector.engine
            act = nc.scalar.engine
            sp = nc.sync.engine

            for i in range(PER_CORE):
                # --- 1. row gather: in[a[k], :] -> M1 ---
                m1 = m1_pool.tile([128, M1_W], DT, tag="m1")
                if coresim_pads:
                    # padded edge/main copies read into the lead/tail pads;
                    # the values only ever land in dump regions, but CoreSim
                    # requires every read to be initialized
                    nc.gpsimd.memset(m1[:, 0:M1_LEAD], 0.0)
                    nc.gpsimd.memset(m1[:, M1_W - PAD:M1_W], 0.0)
                src = AP(images[:].tensor, i * img_elems,
                         [[ROW_ELEMS, H], [1, ROW_ELEMS]])
                gout = m1[:, M1_LEAD:M1_LEAD + 2 * ROW_ELEMS].rearrange(
                    "p (h e) -> p h e", h=2)
                nc.gpsimd.dma_gather(
                    gout, src, idx_t[:, 16 * i:16 * (i + 1)],
                    num_idxs=H, num_idxs_reg=H, elem_size=ROW_ELEMS,
                    queue_num=i % 2, single_packet=False)

                # --- 2. column gather by b: M1 -> Ntile (4 reg-offset copies) ---
                ntile = n_pool.tile([128, N_W], DT, tag="ncg")
                m1t, ntt = m1[:].tensor, ntile[:].tensor
                p_m1 = [M1_W, 128]
                p_n = [N_W, 128]
                # per-image virtual registers; 5 per image (main src/dst,
                # edge src/dst, R flag), loaded per image pair. The R flag
                # branches ONLY the DVE stream: R=0 runs {asc main, desc
                # edge}, R=1 runs {desc main, asc edge} - halving DVE work
                # vs executing all four direction variants.
                if i % 2 == 0:
                    nload = min(2, PER_CORE - i) * 5
                    pair_regs = [nc.alloc_register(dve, f"cg{i}_{j}")
                                 for j in range(nload)]
                    nc.vector.reg_load(
                        pair_regs, par_t[0:1, 5 * i:5 * i + nload])
                dve_regs = pair_regs[5 * (i % 2):5 * (i % 2) + 5]
                with tc.If(bass.RuntimeValue(dve_regs[4]) < 1) as cmp:
                    nc.vector.tensor_copy(
                        AP(ntt, dve_regs[1], [p_n, [N_HSTRIDE, 2], [1, ROW_ELEMS]]),
                        AP(m1t, dve_regs[0], [p_m1, [M1_HSTRIDE, 2], [1, ROW_ELEMS]]))
                    nc.vector.tensor_copy(
                        AP(ntt, dve_regs[3], [p_n, [N_HSTRIDE, 2], [1, 3 * EDGE_PIX]]),
                        AP(m1t, dve_regs[2], [p_m1, [M1_HSTRIDE, 2], [-3, EDGE_PIX], [1, C]]))
                with cmp.Else():
                    nc.vector.tensor_copy(
                        AP(ntt, dve_regs[1], [p_n, [N_HSTRIDE, 2], [1, ROW_ELEMS]]),
                        AP(m1t, dve_regs[0], [p_m1, [M1_HSTRIDE, 2], [-3, W], [1, C]]))
                    nc.vector.tensor_copy(
                        AP(ntt, dve_regs[3], [p_n, [N_HSTRIDE, 2], [1, 3 * EDGE_PIX]]),
                        AP(m1t, dve_regs[2], [p_m1, [M1_HSTRIDE, 2], [1, 3 * EDGE_PIX]]))

                # --- 3. pixel transpose Ntile -> Ttile via PE (exact fp32) ---
                # 3 channel transposes interleave into one strided PSUM tile;
                # a single contiguous copy (on the otherwise-idle scalar
                # engine) moves each [128, 384] block out
                ttile = t_pool.tile([128, 2, ROW_ELEMS], DT, tag="tt")
                for hk in range(2):
                    for hu in range(2):
                        pt = psum_pool.tile([128, 3 * 128], DT, tag="pt")
                        ptt = pt[:].tensor
                        for c in range(C):
                            stat = AP(ntt, N_LEAD + hk * N_HSTRIDE + 3 * (hu * 128) + c,
                                      [p_n, [3, 128]])
                            nc.tensor.transpose(
                                AP(ptt, c, [[3 * 128, 128], [3, 128]]),
                                stat, ident[:])
                        t0 = 3 * (hk * 128)
                        nc.scalar.copy(ttile[:, hu, t0:t0 + 3 * 128], pt[:])

                # --- 4. predicated stores ---
                dram_out = AP(out[:].tensor, i * img_elems,
                              [[ROW_ELEMS, 128], [128 * ROW_ELEMS, 2], [1, ROW_ELEMS]])
                n_src = AP(ntt, N_LEAD, [p_n, [N_HSTRIDE, 2], [1, ROW_ELEMS]])
                cond_n_reg = nc.alloc_register(sp, f"cond_n_{i}")
                cond_t_reg = nc.alloc_register(sp, f"cond_t_{i}")
                cbase = 5 * PER_CORE + 2 * i
                nc.sync.reg_load([cond_n_reg, cond_t_reg],
                                 par_t[0:1, cbase:cbase + 2])
                cn = nc.sync.snap(cond_n_reg, min_val=0, max_val=1)
                ct = nc.sync.snap(cond_t_reg, min_val=0, max_val=1)
                nc.sync.dma_start(dram_out, n_src, cond=cn)
                nc.sync.dma_start(dram_out.copy(), ttile[:], cond=ct)

    nc.finalize()
    _NC_CACHE[key] = nc
    return nc


def _make_in_maps(images, a, b, tr):
    ident = np.eye(128, dtype=np.float32)
    in_maps = []
    for core in range(N_CORES):
        s = core * PER_CORE
        par = np.zeros((1, NPARAM * PER_CORE), np.int32)
        for i in range(PER_CORE):
            par[0, 5 * i:5 * i + 5] = _fit_template(b[s + i])
            par[0, 5 * PER_CORE + 2 * i] = 0 if tr[s + i] else 1
            par[0, 5 * PER_CORE + 2 * i + 1] = 1 if tr[s + i] else 0
        in_maps.append({
            "images": images[s:s + PER_CORE],
            "identity_in": ident,
            "gidx": _pack_gather_idx(a[s:s + PER_CORE]),
            "params": par,
        })
    return in_maps


def kernel(images, xflip_w, xflip_gate, yflip_w, yflip_gate, rot_w, rot_gate,
           trans_w, trans_gate):
    from concourse.bass_utils import run_bass_kernel_spmd

    images = np.ascontiguousarray(np.asarray(images, dtype=np.float32))
    a, b, tr = _derive_maps(xflip_w, xflip_gate, yflip_w, yflip_gate,
                            rot_w, rot_gate, trans_w, trans_gate)
    nc = _build_module()
    in_maps = _make_in_maps(images, a, b, tr)
    res = run_bass_kernel_spmd(nc, in_maps, list(range(N_CORES))).results
    return np.concatenate([res[c]["out"] for c in range(N_CORES)], axis=0)

